# revision 35
# baseline (speedup 1.0000x reference)
"""Fused multi-head attention (B=2, N=2048, C=1024, H=16) on 8 TRN2 NeuronCores.

Sharding: core = (b, g) with b = batch (2) and g = head-group of 4 heads (4).
Each core computes, for its batch and 4 heads:
    qkv slice -> per-head softmax attention -> out-proj partial (row-parallel).
Host sums the 4 per-head-group proj partials per batch and adds b_proj
(the device computes -out/den and the host feeds -w_proj; see below).

Device algorithm (per core), matmuls in bf16:
  phase 1: qkT = (x @ Wqk)^T   [q/k feats on partitions, 2048 tokens]
           v   = x @ Wv        [2048 tokens, 4*64] (+ ones column per head)
  phase 2: per (head pair, 512-row chunk):
           S^T tiles = 64x128 row-tiled matmuls (even head on PE tile (0,0),
             odd head on (64,0); kTp packs both heads' kT with no zero pad)
           expST = exp(S^T/8)  (ScalarE, PSUM->SBUF, [128,1024] grain)
           outT[65, rows] += [v_h|1]^T-matmul expST  (K=128 keys)
             row 64 = softmax denominator (ones column trick)
           -1/den via DVE bit-trick seed + 1 Newton step (max rel err 2.5e-3;
             the [1,512] DVE reciprocal at 8 cyc/elem was 3.3us/head), then
             GpSimd partition-broadcast and a DVE mult into outT
  phase 3: partial = out^T-matmul Wp -> bf16 DMA out (per 128x512 chunk)

Scheduling: the steady state is ScalarE-exp-bound (~2.3us per kc2 step);
qk/v/proj matmuls stream into the PE slack via a fill queue with per-step
pop counts tuned per block. Inputs arrive as 7 staged contiguous DMAs
(host pre-transposes to [partition, kc, cols]); the first block starts
after just q01+k01(keys 0:512)+v(keys 0:256). Each block pre-emits the
next block's first ST+exp before its own last PV so the ACT queue has no
bubble at block boundaries; block-end normalization chains are off the
PE critical path.
"""

import os
from contextlib import ExitStack

import numpy as np

import concourse.bass as bass
import concourse.mybir as mybir
import concourse.tile as tile
from concourse import bacc
from concourse.bass_utils import run_bass_kernel_spmd

B, N, C = 2, 2048, 1024
HC = 4  # heads per core
D = 64
NCORES = 8
KC = C // 128  # 8 contraction chunks for phase 1
SCALE = D**-0.5  # 0.125

# "f32r" (fp32 data, full-rate PE mode), "bf16", or "f32" (4x slower PE)
MM_DT = os.environ.get("ATTN_MM_DT", "bf16")
ST_TILE_POS = os.environ.get("ATTN_ST_TILE_POS", "1") == "1"


def _np_in_dtype():
    if MM_DT == "bf16":
        import ml_dtypes

        return np.dtype(ml_dtypes.bfloat16)
    return np.dtype(np.float32)


def _prep(a):
    """Cast to the device input dtype; for f32r, pre-round to TF32 (RTNE)."""
    a = np.ascontiguousarray(a)
    if MM_DT != "f32r":
        return a.astype(_np_in_dtype())
    u = a.astype(np.float32).view(np.uint32)
    u = (u + 0x0FFF + ((u >> 13) & 1)) & np.uint32(0xFFFFE000)
    return u.view(np.float32)


def build_nc():
    f32 = mybir.dt.float32
    in_dt = {
        "bf16": mybir.dt.bfloat16,
        "f32r": mybir.dt.float32r,
        "f32": mybir.dt.float32,
    }[MM_DT]
    mm = lambda ap: ap  # noqa: E731

    nc = bacc.Bacc("TRN2", target_bir_lowering=False, debug=False, num_devices=NCORES)
    # all inputs host-pretransposed to [partition, kc, cols] so every DMA
    # stage is one fully-contiguous transfer
    xT0_d = nc.dram_tensor("xT0", [128, KC, 512], in_dt, kind="ExternalInput").ap()
    xT1_d = nc.dram_tensor("xT1", [128, KC, 512], in_dt, kind="ExternalInput").ap()
    xT23_d = nc.dram_tensor("xT23", [128, KC, 1024], in_dt, kind="ExternalInput").ap()
    wqkA_d = nc.dram_tensor(
        "wqkA", [128, 2, KC, 128], in_dt, kind="ExternalInput"
    ).ap()
    wqkB_d = nc.dram_tensor("wqkB", [128, KC, 256], in_dt, kind="ExternalInput").ap()
    wv_d = nc.dram_tensor("wv", [128, KC, 256], in_dt, kind="ExternalInput").ap()
    wp_d = nc.dram_tensor("wp", [128, 2, C], in_dt, kind="ExternalInput").ap()
    out_bf = mybir.dt.bfloat16
    out_d = nc.dram_tensor("out", [N, C], out_bf, kind="ExternalOutput").ap()

    with tile.TileContext(nc) as tc:
        with (
            tc.tile_pool(name="const", bufs=1) as const,
            tc.tile_pool(name="ex", bufs=8) as expool,
            tc.tile_pool(name="den", bufs=6) as dpool,
            tc.tile_pool(name="stage", bufs=4) as stage,
            tc.tile_pool(name="stps", bufs=2, space="PSUM") as stps,
            tc.tile_pool(name="pvps", bufs=4, space="PSUM") as pvps,
        ):
            # persistent tiles
            # qkT chunks: 0 = q heads 0,1; 1 = q heads 2,3
            #   (head even -> partitions 0:64, odd -> 64:128)
            # kTp: per-head zero-padded K=128 stationary operand: head even
            #   has kT in rows 0:64 / zeros in 64:128, head odd the reverse,
            #   so a full-128-row matmul against the stacked q chunk
            #   contracts only the matching head's 64 features.
            qkT_sb = const.tile([128, 2, N], in_dt, tag="qkT")
            kTp_sb = const.tile([128, 2, N], in_dt, tag="kTp")
            v_sb = const.tile([128, 16, HC, D + 1], in_dt, tag="v")
            wp_sb = const.tile([128, 2, C], in_dt, tag="wp")
            outT_sb = const.tile([128, 2, N], in_dt, tag="outT")
            xT0_sb = const.tile([128, KC, 512], in_dt, tag="xT0")
            xT1_sb = const.tile([128, KC, 512], in_dt, tag="xT1")
            xT23_sb = const.tile([128, KC, 1024], in_dt, tag="xT23")
            wqkA_sb = const.tile([128, 2, KC, 128], in_dt, tag="wqkA")
            wqkB_sb = const.tile([128, KC, 256], in_dt, tag="wqkB")
            wv_sb = const.tile([128, KC, HC * D], in_dt, tag="wv")

            def xt_ap(kc, c0, c1):
                """xT slice [kc, token cols c0:c1] from the staged tiles."""
                if c1 <= 512:
                    return xT0_sb[:, kc, c0:c1]
                if c1 <= 1024:
                    return xT1_sb[:, kc, c0 - 512 : c1 - 512]
                return xT23_sb[:, kc, c0 - 1024 : c1 - 1024]

            # ---- DMAs, staged so the first attention block can start
            # early; one contiguous transfer per stage. Stage A carries
            # q01+k01 plus the first 512 tokens of xT and wv; the rest
            # streams in underneath block (0,0).
            # q01 / xT0-halves / k01 split so the first qk matmuls begin
            # before the full stage has landed (tile deps are range-tracked)
            nc.sync.dma_start(wqkA_sb[:, 0], wqkA_d[:, 0])
            nc.sync.dma_start(xT0_sb[:, 0:4], xT0_d[:, 0:4])
            nc.sync.dma_start(wqkA_sb[:, 1], wqkA_d[:, 1])
            nc.sync.dma_start(xT0_sb[:, 4:8], xT0_d[:, 4:8])
            nc.sync.dma_start(wv_sb[:], wv_d)
            nc.sync.dma_start(xT1_sb[:], xT1_d)
            nc.sync.dma_start(wqkB_sb[:], wqkB_d)
            nc.sync.dma_start(xT23_sb[:], xT23_d)
            nc.sync.dma_start(wp_sb[:], wp_d)

            # ---- one-time fills (run during the DMA wait) ----
            ones_f32 = const.tile([128, 16, HC, 1], f32, tag="ones")
            nc.vector.memset(ones_f32[:], 1.0)
            nc.vector.tensor_copy(v_sb[:, :, :, D : D + 1], ones_f32[:])

            # ---- emission helpers ----
            def qk_chunk(mf, nt, big=False):
                """One psum of (x @ Wqk)^T: feat chunk mf, token chunk nt.
                wqk feat chunks (host-reordered): 0 = q heads 0,1;
                1 = k heads 0,1; 2 = q heads 2,3; 3 = k heads 2,3.
                big=True routes the psum to the (idle pre-block) stps pool."""
                if big:
                    ps = stps.tile([128, 1024], f32, tag="st", name="st")[:, :512]
                else:
                    ps = pvps.tile([128, 512], f32, tag="pv", name="pv")
                for kc in range(KC):
                    wap = (
                        wqkA_sb[:, mf, kc, :]
                        if mf < 2
                        else wqkB_sb[:, kc, (mf % 2) * 128 : (mf % 2) * 128 + 128]
                    )
                    nc.tensor.matmul(
                        ps,
                        mm(wap),
                        mm(xt_ap(kc, nt * 512, (nt + 1) * 512)),
                        start=(kc == 0),
                        stop=(kc == KC - 1),
                    )
                nts = slice(nt * 512, (nt + 1) * 512)
                if mf % 2 == 0:  # q chunk
                    nc.vector.tensor_copy(qkT_sb[:, mf // 2, nts], ps)
                else:  # k chunk -> kTp[0:64]=even head, [64:128]=odd head
                    hp = (mf - 1) // 2
                    nc.vector.tensor_copy(kTp_sb[0:64, hp, nts], ps[0:64, :])
                    nc.vector.tensor_copy(kTp_sb[64:128, hp, nts], ps[64:128, :])

            def v_chunk(t):
                """One psum of v = x @ Wv for token(=key) chunk t, all heads."""
                ps = pvps.tile([128, 512], f32, tag="pv", name="pv")[:, : HC * D]
                for kc in range(KC):
                    nc.tensor.matmul(
                        ps,
                        mm(xt_ap(kc, t * 128, (t + 1) * 128)),
                        mm(wv_sb[:, kc, :]),
                        start=(kc == 0),
                        stop=(kc == KC - 1),
                    )
                nc.vector.tensor_copy(
                    v_sb[:, t, :, 0:D], ps.rearrange("p (h d) -> p h d", h=HC)
                )

            def proj_chunk(t, nf, scalar_copy=False):
                """partial[t*128:(t+1)*128, nf*512:(nf+1)*512] = out @ Wp."""
                ps = pvps.tile([128, 512], f32, tag="pv", name="pv")
                for c2 in range(2):
                    nc.tensor.matmul(
                        ps,
                        mm(outT_sb[:, c2, t * 128 : (t + 1) * 128]),
                        mm(wp_sb[:, c2, nf * 512 : (nf + 1) * 512]),
                        start=(c2 == 0),
                        stop=(c2 == 1),
                    )
                sg = stage.tile([128, 512], out_bf, tag="sg", name="sg")
                if scalar_copy:
                    nc.scalar.copy(sg, ps)
                else:
                    nc.vector.tensor_copy(sg, ps)
                nc.sync.dma_start(
                    out_d[t * 128 : (t + 1) * 128, nf * 512 : (nf + 1) * 512], sg
                )

            # fill queue: work interleaved into attention blocks' PE slack
            fills = []

            def st_exp(hp, rc, kc2):
                """Emit one step's ST matmuls + exps; return the ex tiles."""
                heads = (2 * hp, 2 * hp + 1)
                stp = {
                    h: stps.tile([128, 1024], f32, tag="st", name="st")
                    for h in heads
                }
                for j in range(2):
                    kc = 2 * kc2 + j
                    for i, h in enumerate(heads):
                        pb = 64 * i  # PE row-tile (0,0) / (64,0)
                        nc.tensor.matmul(
                            stp[h][:, j * 512 : (j + 1) * 512],
                            mm(kTp_sb[pb : pb + 64, hp, kc * 128 : (kc + 1) * 128]),
                            mm(qkT_sb[pb : pb + 64, hp, rc * 512 : (rc + 1) * 512]),
                            start=True,
                            stop=True,
                        )
                exs = {}
                for h in heads:
                    ex = expool.tile([128, 1024], in_dt, tag="ex", name="ex")
                    nc.scalar.activation(
                        ex, stp[h], mybir.ActivationFunctionType.Exp, scale=SCALE
                    )
                    exs[h] = ex
                return exs

            def attention_block(
                hp, rc, sched, head_exs=None, next_se=None, late=0, split_mult=False
            ):
                """ST + exp + PV for head pair hp, 512-row chunk rc; pops
                sched[kc2] fill closures at the top of each kc2 step.
                head_exs: this block's step-0 ST/exp, pre-emitted by the
                previous block so the PE/ACT queues have no boundary bubble.
                next_se: closure emitting the NEXT block's step-0 ST/exp,
                called right after this block's last exp; returned."""
                heads = (2 * hp, 2 * hp + 1)
                pv = {
                    h: pvps.tile([128, 512], f32, tag="pv", name="pv") for h in heads
                }
                nxt = None
                for kc2 in range(8):
                    for _ in range(sched[kc2]):
                        if fills:
                            fills.pop(0)()
                    if kc2 == 0 and head_exs is not None:
                        exs = head_exs
                    else:
                        exs = st_exp(hp, rc, kc2)
                    if kc2 == 7:
                        for _ in range(late):
                            if fills:
                                fills.pop(0)()
                        if next_se is not None:
                            nxt = next_se()
                    for h in heads:
                        for j in range(2):
                            kc = 2 * kc2 + j
                            nc.tensor.matmul(
                                pv[h][: D + 1, :],
                                mm(v_sb[:, kc, h, :]),
                                mm(exs[h][:, j * 512 : (j + 1) * 512]),
                                start=(kc == 0),
                                stop=(kc == 15),
                            )
                # -1/den via bit-trick seed + 1 Newton step on a [2,512]
                # tile (8 cyc/elem DVE reciprocal on a single partition is
                # 3.3us per head; this chain is ~2us for BOTH heads, max rel
                # err 2.5e-3, well inside the 2e-2 budget). The sign flip
                # saves a DVE op; the host negates w_proj to compensate.
                # DVE partition bases must be 32-aligned: heads at rows 0, 32.
                i32 = mybir.dt.int32
                den2 = dpool.tile([33, 512], f32, tag="den2", name="den2")
                for i, h in enumerate(heads):
                    nc.vector.tensor_copy(
                        den2[32 * i : 32 * i + 1, :], pv[h][D : D + 1, :]
                    )
                r0 = dpool.tile([33, 512], f32, tag="r0n", name="r0n")
                # r0 = bits^-1(K - bits(den)) ~= 1/den (max rel err ~5%)
                nc.vector.tensor_scalar(
                    r0[:].bitcast(i32),
                    den2[:].bitcast(i32),
                    -1,
                    0x7EF311C3,
                    op0=mybir.AluOpType.mult,
                    op1=mybir.AluOpType.add,
                )
                tn = dpool.tile([33, 512], f32, tag="tn", name="tn")
                nc.vector.tensor_tensor(
                    out=tn[:], in0=den2[:], in1=r0[:], op=mybir.AluOpType.mult
                )
                # r1 = (tn - 2) * r0 = -(2 - tn)*r0 ~= -1/den
                r1 = dpool.tile([33, 512], f32, tag="r1n", name="r1n")
                nc.vector.scalar_tensor_tensor(
                    out=r1[:],
                    in0=tn[:],
                    scalar=2.0,
                    in1=r0[:],
                    op0=mybir.AluOpType.subtract,
                    op1=mybir.AluOpType.mult,
                )
                # partition_broadcast sources must sit at partition 0;
                # h-even broadcasts straight from r1 row 0 BEFORE the r1b
                # staging copy so it is not queued behind it on DVE sems
                rbcs = {h: dpool.tile([64, 512], f32, tag="rbc", name="rbc")
                        for h in heads}
                nc.gpsimd.partition_broadcast(rbcs[heads[0]], r1[0:1, :])
                r1b = dpool.tile([1, 512], f32, tag="r1b", name="r1b")
                nc.vector.tensor_copy(r1b[:], r1[32:33, :])
                nc.gpsimd.partition_broadcast(rbcs[heads[1]], r1b[:])
                if split_mult:
                    # last block: per-128-token mults so the first proj
                    # chunks of the drained wave can start sooner
                    for tq in range(4):
                        for h in heads:
                            hb = (h % 2) * 64
                            cs = slice(tq * 128, (tq + 1) * 128)
                            nc.vector.tensor_tensor(
                                out=outT_sb[
                                    hb : hb + 64,
                                    hp,
                                    rc * 512 + tq * 128 : rc * 512 + (tq + 1) * 128,
                                ],
                                in0=pv[h][0:D, cs],
                                in1=rbcs[h][:, cs],
                                op=mybir.AluOpType.mult,
                            )
                else:
                    for h in heads:
                        hb = (h % 2) * 64
                        nc.vector.tensor_tensor(
                            out=outT_sb[hb : hb + 64, hp, rc * 512 : (rc + 1) * 512],
                            in0=pv[h][0:D, :],
                            in1=rbcs[h][:],
                            op=mybir.AluOpType.mult,
                        )
                return nxt

            # ---- schedule ----
            def queue_proj(rc):
                # after the last exp ScalarE is idle: for the final (drained)
                # wave, alternate staging copies between DVE and ScalarE
                fills.extend(
                    [
                        lambda t=t, nf=nf: proj_chunk(t, nf, rc == 3 and nf == 1)
                        for t in range(4 * rc, 4 * rc + 4)
                        for nf in range(2)
                    ]
                )

            # up-front minimum to start block (0,0): q01(rc0) and k01(keys
            # 0:512); everything else (v included) streams in as fills.
            qk_chunk(0, 0, big=True)
            qk_chunk(1, 0, big=True)
            se = st_exp(0, 0, 0)  # first exp starts before the v chunks
            v_chunk(0)
            v_chunk(1)
            # block (0,0): v(kc) lands just before its PV(kc) consumer
            # (its write must be EMITTED before the consuming PV's read),
            # k01 chunk nt before the ST step that reads keys nt*512.
            fills.extend(
                [lambda t=t: v_chunk(t) for t in (2, 3)]
                + [lambda: qk_chunk(1, 1)]
                + [lambda t=t: v_chunk(t) for t in (4, 5, 6, 7)]
                + [lambda: qk_chunk(1, 2)]
                + [lambda t=t: v_chunk(t) for t in (8, 9, 10, 11)]
                + [lambda: qk_chunk(1, 3)]
                + [lambda t=t: v_chunk(t) for t in (12, 13, 14, 15)]
            )
            fills.append(lambda: qk_chunk(0, 1))  # q01 rc1 before (0,1)
            se = attention_block(
                0,
                0,
                sched=[0, 2, 3, 2, 3, 2, 3, 0],
                head_exs=se,
                next_se=lambda: st_exp(0, 1, 0),
                late=3,
            )
            fills.extend(
                [
                    lambda: qk_chunk(3, 0),
                    lambda: qk_chunk(3, 1),
                    lambda: qk_chunk(2, 0),
                ]
            )
            se = attention_block(
                0,
                1,
                sched=[1, 1, 1, 0, 0, 0, 0, 0],
                head_exs=se,
                next_se=lambda: st_exp(1, 0, 0),
            )
            fills.extend(
                [
                    lambda: qk_chunk(3, 2),
                    lambda: qk_chunk(3, 3),
                    lambda: qk_chunk(2, 1),
                ]
            )
            fills.append(lambda: qk_chunk(0, 2))  # q01 rc2 before (0,2)
            se = attention_block(
                1,
                0,
                sched=[1, 0, 1, 0, 1, 0, 0, 0],
                head_exs=se,
                next_se=lambda: st_exp(0, 2, 0),
                late=1,
            )
            queue_proj(0)
            se = attention_block(
                0,
                2,
                sched=[0, 0, 1, 1, 1, 1, 0, 0],
                head_exs=se,
                next_se=lambda: st_exp(1, 1, 0),
                late=1,
            )
            fills.append(lambda: qk_chunk(2, 2))
            fills.append(lambda: qk_chunk(0, 3))  # q01 rc3 before (0,3)
            se = attention_block(
                1,
                1,
                sched=[1, 0, 1, 0, 1, 0, 1, 0],
                head_exs=se,
                next_se=lambda: st_exp(0, 3, 0),
                late=1,
            )
            queue_proj(1)
            se = attention_block(
                0,
                3,
                sched=[0, 0, 1, 1, 1, 1, 0, 0],
                head_exs=se,
                next_se=lambda: st_exp(1, 2, 0),
                late=1,
            )
            fills.append(lambda: qk_chunk(2, 3))
            se = attention_block(
                1,
                2,
                sched=[1, 0, 1, 0, 1, 0, 0, 0],
                head_exs=se,
                next_se=lambda: st_exp(1, 3, 0),
                late=1,
            )
            queue_proj(2)
            attention_block(
                1,
                3,
                sched=[0, 0, 2, 1, 2, 1, 1, 0],
                head_exs=se,
                late=1,
                split_mult=True,
            )
            queue_proj(3)
            # drain remaining fill work (proj of rc3)
            while fills:
                fills.pop(0)()
    nc.compile()
    return nc


def _pkc(a):
    """[C_chunkable rows, X] -> [128, rows/128, X] (partition-major)."""
    a = np.asarray(a)
    return np.ascontiguousarray(
        a.reshape(a.shape[0] // 128, 128, a.shape[1]).transpose(1, 0, 2)
    )


def make_in_maps(x, w_qkv, w_proj):
    in_maps = []
    for core in range(NCORES):
        b, g = core // 4, core % 4
        qs = slice(g * 256, (g + 1) * 256)
        xT = np.ascontiguousarray(x[b].T)
        q_cols = w_qkv[:, qs]
        k_cols = w_qkv[:, C + g * 256 : C + (g + 1) * 256]
        in_maps.append(
            {
                "xT0": _prep(_pkc(xT[:, 0:512])),
                "xT1": _prep(_pkc(xT[:, 512:1024])),
                "xT23": _prep(_pkc(xT[:, 1024:2048])),
                "wqkA": _prep(
                    np.stack([_pkc(q_cols[:, :128]), _pkc(k_cols[:, :128])], axis=1)
                ),
                "wqkB": _prep(
                    _pkc(np.concatenate([q_cols[:, 128:], k_cols[:, 128:]], axis=1))
                ),
                "wv": _prep(_pkc(w_qkv[:, 2 * C + g * 256 : 2 * C + (g + 1) * 256])),
                "wp": _prep(_pkc(-w_proj[qs, :])),
            }
        )
    return in_maps


def run_hw(x, w_qkv, w_proj, b_proj, trace=False):
    """Returns (full output [2, 2048, 1024] f32, exec_time_ns or None)."""
    in_maps = make_in_maps(x, w_qkv, w_proj)
    nc = build_nc()
    r = run_bass_kernel_spmd(nc, in_maps, core_ids=list(range(NCORES)), trace=trace)
    full = np.zeros((B, N, C), np.float32)
    for core in range(NCORES):
        full[core // 4] += np.asarray(r.results[core]["out"], np.float32)
    full += np.asarray(b_proj, np.float32)[None, None, :]
    return full, r.exec_time_ns


def kernel(**inputs):
    x = np.asarray(inputs["x"], np.float32)
    w_qkv = np.asarray(inputs["w_qkv"], np.float32)
    w_proj = np.asarray(inputs["w_proj"], np.float32)
    b_proj = np.asarray(inputs["b_proj"], np.float32)
    out, _ = run_hw(x, w_qkv, w_proj, b_proj, trace=False)
    return out

